# revision 1
# baseline (speedup 1.0000x reference)
"""Trainium2 Bass kernel for nn_HadamardTransform: out = value @ (weight + permutation).

Strategy: data-parallel over the 8192 token rows across 8 NeuronCores.
Everything runs in the transposed frame so both matmul operands are
natural-layout (contraction dim on partitions, no on-device transposes):

    O_c[n, m] = sum_k (weight+perm)[k, n] * value.T[k, m_c]   (per core c)

lhsT = W' column panel [128k, 128n] (fp32r), rhs = value.T tile [128k, 512m]
(fp32r), accumulated over 32 k-tiles into PSUM [128n, 512m].  W' is computed
on-device (DVE add of weight and permutation panels).  Host transposes the
gathered O_c back to [rows, n].
"""

import sys

sys.path.insert(0, "/opt/trn_rl_repo")

import numpy as np

import concourse.bacc as bacc
import concourse.bass as bass
import concourse.mybir as mybir
import concourse.tile as tile
from concourse.bass_utils import run_bass_kernel_spmd

ROWS = 8192
N = 4096
N_CORES = 8
MPC = ROWS // N_CORES  # 1024 token rows per core
KT = N // 128  # 32 k-tiles
NB = N // 128  # 32 n-blocks
MC = MPC // 512  # 2 m-chunks

_cache = {}


def build_dense():
    nc = bacc.Bacc("TRN2", target_bir_lowering=False)
    vT = nc.dram_tensor("vT", (N, MPC), mybir.dt.float32r, kind="ExternalInput")
    wgt = nc.dram_tensor("wgt", (N, N), mybir.dt.float32, kind="ExternalInput")
    prm = nc.dram_tensor("prm", (N, N), mybir.dt.float32, kind="ExternalInput")
    o = nc.dram_tensor("o", (N, MPC), mybir.dt.float32, kind="ExternalOutput")

    with tile.TileContext(nc) as tc:
        with (
            tc.tile_pool(name="vt", bufs=1) as vt_pool,
            tc.tile_pool(name="wp", bufs=2) as wp_pool,
            tc.tile_pool(name="pp", bufs=2) as pp_pool,
            tc.tile_pool(name="ps", bufs=4, space="PSUM") as ps_pool,
            tc.tile_pool(name="os", bufs=4) as os_pool,
        ):
            # resident value.T shard: 32 tiles [128, 1024] (16 MB)
            vts = []
            for t in range(KT):
                vt_t = vt_pool.tile([128, MPC], mybir.dt.float32r, tag=f"vt{t}")
                nc.sync.dma_start(out=vt_t, in_=vT[t * 128 : (t + 1) * 128, :])
                vts.append(vt_t)

            for nb in range(NB):
                n0 = nb * 128
                # W' column panel [128 k-part, (kt, j) free] for 128 n-cols
                wp = wp_pool.tile([128, KT, 128], mybir.dt.float32r, tag="wp")
                pp = pp_pool.tile([128, KT, 128], mybir.dt.float32, tag="pp")
                wsrc = wgt[:, n0 : n0 + 128].rearrange("(kt p) j -> p kt j", p=128)
                psrc = prm[:, n0 : n0 + 128].rearrange("(kt p) j -> p kt j", p=128)
                nc.sync.dma_start(out=wp[:, :, :].bitcast(mybir.dt.float32), in_=wsrc)
                nc.sync.dma_start(out=pp, in_=psrc)
                nc.vector.tensor_tensor(
                    out=wp[:, :, :],
                    in0=wp[:, :, :].bitcast(mybir.dt.float32),
                    in1=pp[:, :, :],
                    op=mybir.AluOpType.add,
                )
                for mc in range(MC):
                    ps = ps_pool.tile([128, 512], mybir.dt.float32, tag="ps")
                    for kt in range(KT):
                        nc.tensor.matmul(
                            out=ps[:, :],
                            lhsT=wp[:, kt, :],
                            rhs=vts[kt][:, mc * 512 : (mc + 1) * 512],
                            start=(kt == 0),
                            stop=(kt == KT - 1),
                        )
                    ot = os_pool.tile([128, 512], mybir.dt.float32, tag="os")
                    nc.scalar.copy(out=ot[:, :], in_=ps[:, :])
                    nc.sync.dma_start(
                        out=o[n0 : n0 + 128, mc * 512 : (mc + 1) * 512], in_=ot
                    )
    nc.compile()
    return nc


def make_in_maps(value, weight, permutation):
    vT = np.ascontiguousarray(value.T)  # [N, ROWS]
    w = np.ascontiguousarray(weight, dtype=np.float32)
    p = np.ascontiguousarray(permutation, dtype=np.float32)
    in_maps = []
    for c in range(N_CORES):
        in_maps.append(
            {
                "vT": np.ascontiguousarray(vT[:, c * MPC : (c + 1) * MPC]),
                "wgt": w,
                "prm": p,
            }
        )
    return in_maps


def kernel(value, weight, permutation):
    value = np.asarray(value, dtype=np.float32)
    weight = np.asarray(weight, dtype=np.float32)
    permutation = np.asarray(permutation, dtype=np.float32)
    src = check_structure(weight, permutation)
    if src is not None:
        if "had" not in _cache:
            _cache["had"] = build_hadamard()
        nc = _cache["had"]
        in_maps = make_in_maps_h(value, src)
    else:
        if "dense" not in _cache:
            _cache["dense"] = build_dense()
        nc = _cache["dense"]
        in_maps = make_in_maps(value, weight, permutation)
    res = run_bass_kernel_spmd(nc, in_maps, core_ids=list(range(N_CORES)))
    out = np.concatenate(
        [np.ascontiguousarray(res.results[c]["o"].T) for c in range(N_CORES)], axis=0
    )
    return out


# ---------------- structured (Hadamard) path ----------------

I1 = 4          # high radix (H4 butterflies on DVE)
B = N // I1     # 1024-point transform on the PE
KS = B // 128   # 8 k-subtiles per i1


def _hadamard_pm1(n):
    idx = np.arange(n, dtype=np.int64)
    m = idx[:, None] & idx[None, :]
    pop = np.zeros_like(m)
    for _ in range(int(np.log2(n))):
        pop += m & 1
        m >>= 1
    return np.where(pop % 2 == 0, 1.0, -1.0).astype(np.float32)


def check_structure(weight, permutation):
    """weight must be the scaled Sylvester Hadamard, permutation one-hot."""
    H = _hadamard_pm1(N) / np.sqrt(np.float32(N))
    if not np.array_equal(weight, H):
        return None
    src = np.argmax(permutation, axis=0).astype(np.int32)
    ok = (
        permutation[src, np.arange(N)].min() == 1.0
        and permutation.sum() == N
        and np.abs(permutation).sum() == N
    )
    return src if ok else None


def build_hadamard(reps=1):
    nc = bacc.Bacc("TRN2", target_bir_lowering=False)
    vT = nc.dram_tensor("vT", (N, MPC), mybir.dt.float32r, kind="ExternalInput")
    hc = nc.dram_tensor("hc", (B, B), mybir.dt.float32r, kind="ExternalInput")
    gidx = nc.dram_tensor("gidx", (N, 1), mybir.dt.int32, kind="ExternalInput")
    o = nc.dram_tensor("o", (N, MPC), mybir.dt.float32, kind="ExternalOutput")

    J2B = B // 128  # 8 j2 blocks

    with tile.TileContext(nc) as tc:
        with (
            tc.tile_pool(name="h", bufs=1) as h_pool,
            tc.tile_pool(name="gi", bufs=1) as gi_pool,
            tc.tile_pool(name="vt", bufs=1) as vt_pool,
            tc.tile_pool(name="ps", bufs=4, space="PSUM") as ps_pool,
            tc.tile_pool(name="u", bufs=2) as u_pool,
            tc.tile_pool(name="t", bufs=2) as t_pool,
            tc.tile_pool(name="g", bufs=2) as g_pool,
            tc.tile_pool(name="ob", bufs=2) as ob_pool,
        ):
            hts = []
            for ks in range(KS):
                ht = h_pool.tile([128, B], mybir.dt.float32r, tag=f"h{ks}")
                nc.sync.dma_start(out=ht, in_=hc[ks * 128 : (ks + 1) * 128, :])
                hts.append(ht)
            gi = gi_pool.tile([128, NB], mybir.dt.int32, tag="gi")
            nc.sync.dma_start(
                out=gi, in_=gidx[:, 0].rearrange("(nb p) -> p nb", p=128)
            )

            for rep in range(reps):
              for mc in range(MC):
                m0 = mc * 512
                vts = []
                for kt in range(KT):
                    vt_t = vt_pool.tile(
                        [128, 512], mybir.dt.float32r, tag=f"vt{kt}"
                    )
                    nc.sync.dma_start(
                        out=vt_t, in_=vT[kt * 128 : (kt + 1) * 128, m0 : m0 + 512]
                    )
                    vts.append(vt_t)
                for j2b in range(J2B):
                    us = []
                    for i1 in range(I1):
                        ps = ps_pool.tile([128, 512], mybir.dt.float32, tag="ps")
                        for ks in range(KS):
                            nc.tensor.matmul(
                                out=ps[:, :],
                                lhsT=hts[ks][:, j2b * 128 : (j2b + 1) * 128],
                                rhs=vts[i1 * KS + ks][:, :],
                                start=(ks == 0),
                                stop=(ks == KS - 1),
                            )
                        u = u_pool.tile([128, 512], mybir.dt.float32, tag=f"u{i1}")
                        nc.scalar.copy(out=u[:, :], in_=ps[:, :])
                        us.append(u)
                    ts = [
                        t_pool.tile(
                            [128, 512], mybir.dt.float32, tag=f"t{i}", name=f"t{i}"
                        )
                        for i in range(I1)
                    ]
                    add, sub = mybir.AluOpType.add, mybir.AluOpType.subtract
                    nc.vector.tensor_tensor(out=ts[0][:, :], in0=us[0][:, :], in1=us[1][:, :], op=add)
                    nc.vector.tensor_tensor(out=ts[1][:, :], in0=us[0][:, :], in1=us[1][:, :], op=sub)
                    nc.vector.tensor_tensor(out=ts[2][:, :], in0=us[2][:, :], in1=us[3][:, :], op=add)
                    nc.vector.tensor_tensor(out=ts[3][:, :], in0=us[2][:, :], in1=us[3][:, :], op=sub)
                    pairs = [(0, 2, add), (1, 3, add), (0, 2, sub), (1, 3, sub)]
                    for j1, (a, b_, op) in enumerate(pairs):
                        nb = j1 * J2B + j2b
                        ob = ob_pool.tile([128, 512], mybir.dt.float32, tag=f"ob{j1}")
                        nc.vector.tensor_tensor(
                            out=ob[:, :], in0=ts[a][:, :], in1=ts[b_][:, :], op=op
                        )
                        g = g_pool.tile([128, 512], mybir.dt.float32, tag=f"g{j1}")
                        nc.gpsimd.indirect_dma_start(
                            out=g[:, :],
                            out_offset=None,
                            in_=vT[:, :].bitcast(mybir.dt.float32),
                            in_offset=bass.IndirectOffsetOnAxis(
                                ap=gi[:, nb : nb + 1], axis=0
                            ),
                            element_offset=m0,
                        )
                        nc.vector.tensor_tensor(
                            out=ob[:, :], in0=ob[:, :], in1=g[:, :], op=add
                        )
                        nc.sync.dma_start(
                            out=o[nb * 128 : (nb + 1) * 128, m0 : m0 + 512],
                            in_=ob[:, :],
                        )
    nc.compile()
    return nc


def make_in_maps_h(value, src):
    vT = np.ascontiguousarray(value.T)
    Hs = np.ascontiguousarray(_hadamard_pm1(B) / 64.0)
    gidx = src.reshape(N, 1)
    in_maps = []
    for c in range(N_CORES):
        in_maps.append(
            {
                "vT": np.ascontiguousarray(vT[:, c * MPC : (c + 1) * MPC]),
                "hc": Hs,
                "gidx": gidx,
            }
        )
    return in_maps



# revision 28
# speedup vs baseline: 366.0975x; 366.0975x over previous
"""Trainium2 Bass kernel for nn_HadamardTransform: out = value @ (weight + permutation).

Strategy: data-parallel over the 8192 token rows across 8 NeuronCores.
Structured path exploits weight = Sylvester Hadamard (H4096 = H8 (x) H512)
and permutation = one-hot:

    out.T[j1*512+j0, m] = sum_i1 H8[i1,j1] * U_i1[j0, m]  +  vT[src[j], m]
    U_i1 = (H512/64) @ vT[i1*512:(i1+1)*512, :]

Per core (1024 rows = columns of the transposed frame), all bf16:
  - PE: U via matmuls (lhsT = H512 panel bf16, rhs = resident vT tiles bf16)
  - ACT: PSUM->SBUF evacuation with fp32->bf16 cast (2 x i1 per copy)
  - DVE: radix-8 butterfly, 8 lanes fused per op via strided APs
    ([128,4,1024] per instruction) + the permutation add
  - SWDGE: permutation rows pre-gathered from DRAM early (decoupled from
    the butterflies so the Q7 descriptor generation overlaps compute),
    4 row-blocks (512 rows) per indirect DMA
  - out written bf16 in 8 batched DMAs; host upcasts to fp32
"""

import sys

sys.path.insert(0, "/opt/trn_rl_repo")

import numpy as np

import concourse.bacc as bacc
import concourse.bass as bass
import concourse.mybir as mybir
import concourse.tile as tile
from concourse.bass_utils import run_bass_kernel_spmd

ROWS = 8192
N = 4096
N_CORES = 8
MPC = ROWS // N_CORES  # 1024 token rows per core
KT = N // 128  # 32 k-tiles of vT
NB = N // 128  # 32 output row blocks

BF16 = mybir.dt.bfloat16
NP_BF16 = mybir.dt.np(mybir.dt.bfloat16)

_cache = {}


# ---------------- dense fallback (arbitrary weight/permutation) ----------------

def build_dense():
    nc = bacc.Bacc("TRN2", target_bir_lowering=False)
    vT = nc.dram_tensor("vT", (N, MPC), mybir.dt.float32r, kind="ExternalInput")
    wgt = nc.dram_tensor("wgt", (N, N), mybir.dt.float32, kind="ExternalInput")
    prm = nc.dram_tensor("prm", (N, N), mybir.dt.float32, kind="ExternalInput")
    o = nc.dram_tensor("o", (N, MPC), mybir.dt.float32, kind="ExternalOutput")

    MC = MPC // 512

    with tile.TileContext(nc) as tc:
        with (
            tc.tile_pool(name="vt", bufs=1) as vt_pool,
            tc.tile_pool(name="wp", bufs=2) as wp_pool,
            tc.tile_pool(name="pp", bufs=2) as pp_pool,
            tc.tile_pool(name="ps", bufs=4, space="PSUM") as ps_pool,
            tc.tile_pool(name="os", bufs=4) as os_pool,
        ):
            vts = []
            for t in range(KT):
                vt_t = vt_pool.tile([128, MPC], mybir.dt.float32r, tag=f"vt{t}")
                nc.sync.dma_start(out=vt_t, in_=vT[t * 128 : (t + 1) * 128, :])
                vts.append(vt_t)

            for nb in range(NB):
                n0 = nb * 128
                wp = wp_pool.tile([128, KT, 128], mybir.dt.float32r, tag="wp")
                pp = pp_pool.tile([128, KT, 128], mybir.dt.float32, tag="pp")
                wsrc = wgt[:, n0 : n0 + 128].rearrange("(kt p) j -> p kt j", p=128)
                psrc = prm[:, n0 : n0 + 128].rearrange("(kt p) j -> p kt j", p=128)
                nc.sync.dma_start(out=wp[:, :, :].bitcast(mybir.dt.float32), in_=wsrc)
                nc.sync.dma_start(out=pp, in_=psrc)
                nc.vector.tensor_tensor(
                    out=wp[:, :, :],
                    in0=wp[:, :, :].bitcast(mybir.dt.float32),
                    in1=pp[:, :, :],
                    op=mybir.AluOpType.add,
                )
                for mc in range(MC):
                    ps = ps_pool.tile([128, 512], mybir.dt.float32, tag="ps")
                    for kt in range(KT):
                        nc.tensor.matmul(
                            out=ps[:, :],
                            lhsT=wp[:, kt, :],
                            rhs=vts[kt][:, mc * 512 : (mc + 1) * 512],
                            start=(kt == 0),
                            stop=(kt == KT - 1),
                        )
                    ot = os_pool.tile([128, 512], mybir.dt.float32, tag="os")
                    nc.scalar.copy(out=ot[:, :], in_=ps[:, :])
                    nc.sync.dma_start(
                        out=o[n0 : n0 + 128, mc * 512 : (mc + 1) * 512], in_=ot
                    )
    nc.compile()
    return nc


def make_in_maps(value, weight, permutation):
    vT = np.ascontiguousarray(value.T)
    w = np.ascontiguousarray(weight, dtype=np.float32)
    p = np.ascontiguousarray(permutation, dtype=np.float32)
    in_maps = []
    for c in range(N_CORES):
        in_maps.append(
            {
                "vT": np.ascontiguousarray(vT[:, c * MPC : (c + 1) * MPC]),
                "wgt": w,
                "prm": p,
            }
        )
    return in_maps


# ---------------- structured (Hadamard) path ----------------

I1 = 8          # radix (butterflies on DVE)
B = N // I1     # 512-point transform on the PE
KS = B // 128   # 4 k-subtiles per i1
JB = B // 128   # 4 j0 blocks


def _hadamard_pm1(n):
    idx = np.arange(n, dtype=np.int64)
    m = idx[:, None] & idx[None, :]
    pop = np.zeros_like(m)
    for _ in range(int(np.log2(n))):
        pop += m & 1
        m >>= 1
    return np.where(pop % 2 == 0, 1.0, -1.0).astype(np.float32)


def check_structure(weight, permutation):
    """weight must be the scaled Sylvester Hadamard, permutation one-hot."""
    H = _hadamard_pm1(N) / np.sqrt(np.float32(N))
    if not np.array_equal(weight, H):
        return None
    src = np.argmax(permutation, axis=0).astype(np.int32)
    ok = (
        permutation[src, np.arange(N)].min() == 1.0
        and permutation.sum() == N
        and np.abs(permutation).sum() == N
    )
    return src if ok else None


def build_hadamard(reps=1):
    nc = bacc.Bacc("TRN2", target_bir_lowering=False)
    vT = nc.dram_tensor("vT", (N, MPC), BF16, kind="ExternalInput")
    hc = nc.dram_tensor("hc", (B, B), BF16, kind="ExternalInput")
    # gidx[p, jb*8 + half*4 + q] = src[(half*4+q)*512 + jb*128 + p]
    gidx = nc.dram_tensor("gidx", (128, NB), mybir.dt.int32, kind="ExternalInput")
    o = nc.dram_tensor("o", (N, MPC), BF16, kind="ExternalOutput")

    add, sub = mybir.AluOpType.add, mybir.AluOpType.subtract
    f32 = mybir.dt.float32

    # output view: row = j1*512 + jb*128 + p
    oview = o.rearrange("(j1 jb p) m -> p j1 jb m", j1=I1, jb=JB, p=128)
    vtv = vT.rearrange("(kt p) m -> p kt m", p=128)
    hcv = hc.rearrange("(ks p) j -> p ks j", p=128)

    with tile.TileContext(nc) as tc:
        with (
            tc.tile_pool(name="h", bufs=1) as h_pool,
            tc.tile_pool(name="gi", bufs=1) as gi_pool,
            tc.tile_pool(name="vt", bufs=1) as vt_pool,
            tc.tile_pool(name="ps", bufs=2, space="PSUM") as ps_pool,
            tc.tile_pool(name="u", bufs=2) as u_pool,
            tc.tile_pool(name="t1", bufs=1) as t1_pool,
            tc.tile_pool(name="t2", bufs=1) as t2_pool,
            tc.tile_pool(name="g", bufs=2) as g_pool,
            tc.tile_pool(name="ob", bufs=2) as ob_pool,
        ):
            ht = h_pool.tile([128, KS, B], BF16, tag="h")
            nc.sync.dma_start(out=ht, in_=hcv)
            # PE warmup: lift the HAM clock-gate to 8/8 while inputs stream in
            warm = ps_pool.tile([128, 2, MPC], mybir.dt.float32, tag="ps")
            for w in range(24):
                nc.tensor.matmul(
                    out=warm[:, 0, 0:512],
                    lhsT=ht[:, 0, 0:128],
                    rhs=ht[:, 0, :],
                    start=True,
                    stop=True,
                )
            vt = vt_pool.tile([128, KT, MPC], BF16, tag="vt")
            for g8 in range(8):
                nc.sync.dma_start(
                    out=vt[:, g8 * 4 : (g8 + 1) * 4, :],
                    in_=vtv[:, g8 * 4 : (g8 + 1) * 4, :],
                )
            # gi intentionally loads after vt (same HWDGE ring) so the
            # permutation gathers don't contend with the input stream
            gi = gi_pool.tile([128, JB, 2, 4], mybir.dt.int32, tag="gi")
            nc.sync.dma_start(out=gi[:, :, :, :], in_=gidx[:, :])

            def issue_gathers(jb, gts):
                for half in range(2):
                    gt = g_pool.tile(
                        [128, 4 * MPC], BF16,
                        tag=f"g{half}", name=f"g{half}_{jb}",
                    )
                    for q in range(4):
                        nc.gpsimd.indirect_dma_start(
                            out=gt[:, q * MPC : (q + 1) * MPC],
                            out_offset=None,
                            in_=vT[:, :],
                            in_offset=bass.IndirectOffsetOnAxis(
                                ap=gi[:, jb, half, q : q + 1], axis=0
                            ),
                        )
                    gts[(jb, half)] = gt

            for rep in range(reps):
                # permutation gathers run on Q7 decoupled from compute; issue
                # two jb's worth ahead (g pool bufs=2) so the Q7 stream never
                # waits at its queue head on a not-yet-issued consumer.
                gts = {}
                issue_gathers(0, gts)
                issue_gathers(1, gts)

                for jb in range(JB):
                    # u[b2, b1, b0] = U_{i1}, i1 = b2*4 + b1*2 + b0
                    ut = u_pool.tile([128, 2, 2, 2, MPC], BF16, tag="u")
                    for grp in range(4):  # (b2, b1); b0 = inner pair
                        b2, b1 = grp >> 1, grp & 1
                        ps = ps_pool.tile([128, 2, MPC], f32, tag="ps")
                        for b0 in range(2):
                            i1 = grp * 2 + b0
                            for mc in range(2):
                                for ks in range(KS):
                                    nc.tensor.matmul(
                                        out=ps[:, b0, mc * 512 : (mc + 1) * 512],
                                        lhsT=ht[:, ks, jb * 128 : (jb + 1) * 128],
                                        rhs=vt[
                                            :, i1 * KS + ks, mc * 512 : (mc + 1) * 512
                                        ],
                                        start=(ks == 0),
                                        stop=(ks == KS - 1),
                                    )
                        nc.scalar.copy(out=ut[:, b2, b1, :, :], in_=ps[:, :, :])
                    # radix-8 butterfly: 4 lanes fused per op (wide strided
                    # APs measured fastest; the DVE bf16 rate is flat at
                    # ~0.56 ns/partition-elem regardless of shaping)
                    t1 = t1_pool.tile([128, 2, 2, 2, MPC], BF16, tag="t1")
                    t2 = t2_pool.tile([128, 2, 2, 2, MPC], BF16, tag="t2")
                    # stage 1 (pairs differ in b0): t1[b2,b1,a0]
                    nc.vector.tensor_tensor(out=t1[:, :, :, 0, :], in0=ut[:, :, :, 0, :], in1=ut[:, :, :, 1, :], op=add)
                    nc.vector.tensor_tensor(out=t1[:, :, :, 1, :], in0=ut[:, :, :, 0, :], in1=ut[:, :, :, 1, :], op=sub)
                    # stage 2 (pairs differ in b1): t2[b2,a1,a0]
                    nc.vector.tensor_tensor(out=t2[:, :, 0, :, :], in0=t1[:, :, 0, :, :], in1=t1[:, :, 1, :, :], op=add)
                    nc.vector.tensor_tensor(out=t2[:, :, 1, :, :], in0=t1[:, :, 0, :, :], in1=t1[:, :, 1, :, :], op=sub)
                    # stage 3 (pairs differ in b2) -> t1 reused: [a2][a1,a0]
                    nc.vector.tensor_tensor(out=t1[:, 0, :, :, :], in0=t2[:, 0, :, :, :], in1=t2[:, 1, :, :, :], op=add)
                    nc.vector.tensor_tensor(out=t1[:, 1, :, :, :], in0=t2[:, 0, :, :, :], in1=t2[:, 1, :, :, :], op=sub)
                    # permutation add + output write, per half (a2)
                    for half in range(2):
                        ob = ob_pool.tile(
                            [128, 4 * MPC], BF16,
                            tag=f"ob{half}", name=f"ob{half}_{jb}",
                        )
                        nc.vector.tensor_tensor(
                            out=ob[:, :],
                            in0=t1[:, half, :, :, :],
                            in1=gts[(jb, half)][:, :],
                            op=add,
                        )
                        nc.sync.dma_start(
                            out=oview[:, half * 4 : (half + 1) * 4, jb, :],
                            in_=ob[:, :],
                        )
                    if jb + 2 < JB:
                        issue_gathers(jb + 2, gts)
    nc.compile()
    return nc


def make_in_maps_h(value, src):
    vT = np.ascontiguousarray(value.T)
    Hs = np.ascontiguousarray((_hadamard_pm1(B) / 64.0).astype(NP_BF16))
    # gidx[p, jb*8 + half*4 + q] = src[(half*4+q)*512 + jb*128 + p]
    gidx = np.ascontiguousarray(
        src.reshape(2, 4, JB, 128).transpose(3, 2, 0, 1).reshape(128, NB)
    ).astype(np.int32)
    in_maps = []
    for c in range(N_CORES):
        in_maps.append(
            {
                "vT": np.ascontiguousarray(vT[:, c * MPC : (c + 1) * MPC]).astype(
                    NP_BF16
                ),
                "hc": Hs,
                "gidx": gidx,
            }
        )
    return in_maps


def kernel(value, weight, permutation):
    value = np.asarray(value, dtype=np.float32)
    weight = np.asarray(weight, dtype=np.float32)
    permutation = np.asarray(permutation, dtype=np.float32)
    src = check_structure(weight, permutation)
    if src is not None:
        if "had" not in _cache:
            _cache["had"] = build_hadamard()
        nc = _cache["had"]
        in_maps = make_in_maps_h(value, src)
        res = run_bass_kernel_spmd(nc, in_maps, core_ids=list(range(N_CORES)))
        out = np.concatenate(
            [
                np.ascontiguousarray(res.results[c]["o"].astype(np.float32).T)
                for c in range(N_CORES)
            ],
            axis=0,
        )
        return out
    if "dense" not in _cache:
        _cache["dense"] = build_dense()
    nc = _cache["dense"]
    in_maps = make_in_maps(value, weight, permutation)
    res = run_bass_kernel_spmd(nc, in_maps, core_ids=list(range(N_CORES)))
    out = np.concatenate(
        [np.ascontiguousarray(res.results[c]["o"].T) for c in range(N_CORES)], axis=0
    )
    return out
